# revision 4
# baseline (speedup 1.0000x reference)
"""Causal attention with ALiBi for Trainium2, tensor-parallel over heads x
data-parallel over batch (8 NeuronCores).

Problem: B=4, S=2048, D=2048, NH=16, HD=128, fp32.
  q/k/v = x @ Wq/Wk/Wv ; scores = q k^T / sqrt(HD) + alibi ; causal softmax ;
  out = (probs @ v) @ Wo

Sharding: core (b, g) handles batch b and head group g (8 heads).
  Per core: x[b] (2048x2048), Wq/Wk/Wv[:, g*1024:(g+1)*1024], Wo[g*1024:...].
  Each core returns out_partial^T; host sums the two head-group partials per
  batch and transposes back.

On-core layout (all matmul operands float32r = single-pass fp32 on the PE,
~1.7e-4 rel err, 4x faster than full fp32):
  XT  = x^T        [128(d_inner), 16(d_chunk), 2048(s)]   (PE transpose of x)
  QT_h/KT_h = q^T  [128(hd), 2048(s)] per head
  V_h              [128(k_inner), 16(k_chunk), 128(hd)]
  scores^T blocks  [128(k), 512(q)] = KT_chunk^T @ QT_tile
  softmax: exp((scores*scale + shift[q]) + alibi[k]) where shift[q] =
  slope*(S-1-q) cancels in the softmax but keeps exponents in range (the
  reference subtracts the row max; the ALiBi ramp dominates that max).
  Causal mask: gpsimd affine_select zeroes k>q prefix of diagonal blocks.
  sums via ones-column matmul (partition reduction), recip broadcast via
  ones-row matmul, O^T = V^T @ P^T accumulated in PSUM per q-tile.
  out^T = Wo_g^T @ O^T accumulated over the 8 heads.
"""

import math

import numpy as np

B, S, D, NH = 4, 2048, 2048, 16
HD = D // NH            # 128
NHG = NH // 2           # heads per core
DC = D // 128           # 16 d-chunks
QT_TILES = S // 512     # 4 q tiles
SCALE = 1.0 / math.sqrt(HD)

_cache = {}


def _get_slopes(n):
    def pow2(n):
        start = 2 ** (-(2 ** (-(math.log2(n) - 3))))
        return [start * start**i for i in range(n)]

    if math.log2(n).is_integer():
        return pow2(n)
    c = 2 ** math.floor(math.log2(n))
    return pow2(c) + _get_slopes(2 * c)[0::2][: n - c]


def _build():
    import concourse.bacc as bacc
    import concourse.mybir as mybir
    import concourse.tile as tile
    from concourse.bass import ts

    f32 = mybir.dt.float32
    f32r = mybir.dt.float32r
    Exp = mybir.ActivationFunctionType.Exp

    nc = bacc.Bacc()
    x_in = nc.declare_dram_parameter("x", [S, D], f32r, isOutput=False)
    wq_in = nc.declare_dram_parameter("wq", [D, NHG * HD], f32r, isOutput=False)
    wk_in = nc.declare_dram_parameter("wk", [D, NHG * HD], f32r, isOutput=False)
    wv_in = nc.declare_dram_parameter("wv", [D, NHG * HD], f32r, isOutput=False)
    wo_in = nc.declare_dram_parameter("wo", [NHG * HD, D], f32r, isOutput=False)
    # alibi_k[p, h*16+kc] = -slope_h * (S-1 - (kc*128+p))
    alibi_k_in = nc.declare_dram_parameter("alibi_k", [128, NHG * DC], f32,
                                           isOutput=False)
    # alibi_q[h, q] = +slope_h * (S-1 - q)   (per-query shift)
    alibi_q_in = nc.declare_dram_parameter("alibi_q", [NHG, S], f32,
                                           isOutput=False)
    ones_col_in = nc.declare_dram_parameter("ones_col", [128, 1], f32r,
                                            isOutput=False)
    ones_row_in = nc.declare_dram_parameter("ones_row", [1, 128], f32r,
                                            isOutput=False)
    ident_in = nc.declare_dram_parameter("ident", [128, 128], f32r,
                                         isOutput=False)
    outT = nc.declare_dram_parameter("outT", [D, S], f32, isOutput=True)

    ot_scratch = nc.dram_tensor("ot_scratch", [NHG, 128, S], f32r)

    with tile.TileContext(nc) as tc:
        with (
            tc.tile_pool(name="consts", bufs=1) as pc,
            tc.tile_pool(name="psA", bufs=2, space="PSUM") as psA,
            tc.tile_pool(name="psB", bufs=1, space="PSUM") as psB,
        ):
            alibi_sb = pc.tile([128, NHG * DC], f32, name="alibi_sb")
            ones_col = pc.tile([128, 1], f32r, name="ones_col_sb")
            ones_row = pc.tile([1, 128], f32r, name="ones_row_sb")
            ident_r = pc.tile([128, 128], f32r, name="ident_sb")
            nc.sync.dma_start(alibi_sb[:], alibi_k_in[:])
            nc.sync.dma_start(ones_col[:], ones_col_in[:])
            nc.sync.dma_start(ones_row[:], ones_row_in[:])
            nc.sync.dma_start(ident_r[:], ident_in[:])

            with tc.tile_pool(name="xt", bufs=1) as pxt:
                XT = pxt.tile([128, DC, S], f32r, name="XT")

                # ---- stage 1: transpose x into XT ----
                with tc.tile_pool(name="xload", bufs=2) as px:
                    for sc in range(S // 128):
                        x_sb = px.tile([128, D], f32r, tag="xb", name="x_sb")
                        nc.sync.dma_start(x_sb[:], x_in[ts(sc, 128), :])
                        for dc in range(DC):
                            ptr = psA.tile([128, 512], f32, tag="pp",
                                           name="ptr")
                            nc.tensor.matmul(ptr[:, :128],
                                             x_sb[:, ts(dc, 128)],
                                             ident_r[:],
                                             start=True, stop=True)
                            nc.vector.tensor_copy(
                                XT[:, dc, ts(sc, 128)], ptr[:, :128])

                # ---- stages 2+3: per-head projections + attention ----
                with (
                    tc.tile_pool(name="wp", bufs=2) as pw,
                    tc.tile_pool(name="qkv", bufs=1) as pq,
                    tc.tile_pool(name="att", bufs=2) as pa,
                    tc.tile_pool(name="small", bufs=1) as psm,
                ):
                    for h in range(NHG):
                        qt_sb = pq.tile([128, S], f32r, tag="QT", name="qt_sb")
                        kt_sb = pq.tile([128, S], f32r, tag="KT", name="kt_sb")
                        vt_sb = pq.tile([128, S], f32r, tag="VT", name="vt_sb")
                        v_sb = pq.tile([128, DC, HD], f32r, tag="V",
                                       name="v_sb")

                        for w_in, dst in ((wq_in, qt_sb), (wk_in, kt_sb),
                                          (wv_in, vt_sb)):
                            w_sb = pw.tile([128, DC, HD], f32r, tag="w",
                                           name="w_sb")
                            nc.sync.dma_start(
                                w_sb[:],
                                w_in[:, ts(h, HD)].rearrange(
                                    "(dc p) f -> p dc f", p=128))
                            for st in range(QT_TILES):
                                pp = psA.tile([128, 512], f32, tag="pp",
                                              name="pp")
                                for dc in range(DC):
                                    nc.tensor.matmul(
                                        pp[:], w_sb[:, dc, :],
                                        XT[:, dc, ts(st, 512)],
                                        start=(dc == 0), stop=(dc == DC - 1))
                                nc.vector.tensor_copy(dst[:, ts(st, 512)],
                                                      pp[:])

                        # V = VT^T, chunk by chunk
                        for kc in range(DC):
                            pp = psA.tile([128, 512], f32, tag="pp", name="pp")
                            nc.tensor.matmul(pp[:, :128], vt_sb[:, ts(kc, 128)],
                                             ident_r[:], start=True, stop=True)
                            nc.vector.tensor_copy(v_sb[:, kc, :], pp[:, :128])

                        # attention for head h
                        for qt in range(QT_TILES):
                            nkc = 4 * (qt + 1)
                            shift_sb = psm.tile([128, 512], f32, tag="shift",
                                                name="shift_sb")
                            nc.sync.dma_start(
                                shift_sb[:],
                                alibi_q_in[h, ts(qt, 512)]
                                .partition_broadcast(128))
                            pot = psA.tile([128, 512], f32, tag="pot",
                                           name="pot")
                            psums = psB.tile([1, 512], f32, tag="psums",
                                             name="psums")
                            for kc in range(nkc):
                                pst = psA.tile([128, 512], f32, tag="pst",
                                               name="pst")
                                nc.tensor.matmul(pst[:], kt_sb[:, ts(kc, 128)],
                                                 qt_sb[:, ts(qt, 512)],
                                                 start=True, stop=True)
                                t1 = pa.tile([128, 512], f32, tag="t1",
                                             name="t1")
                                nc.vector.scalar_tensor_tensor(
                                    t1[:], pst[:], SCALE, shift_sb[:],
                                    mybir.AluOpType.mult,
                                    mybir.AluOpType.add)
                                e_sb = pa.tile([128, 512], f32r, tag="e",
                                               name="e_sb")
                                col = h * DC + kc
                                nc.scalar.activation(
                                    e_sb[:], t1[:], Exp,
                                    bias=alibi_sb[:, col:col + 1], scale=1.0)
                                if kc >= 4 * qt:
                                    # zero the masked (k > q) prefix
                                    # keep where qf - kp - r >= 0 (k <= q)
                                    nc.gpsimd.affine_select(
                                        e_sb[:], e_sb[:],
                                        pattern=[[1, 512]],
                                        compare_op=mybir.AluOpType.is_ge,
                                        fill=0.0,
                                        base=-(128 * kc - 512 * qt),
                                        channel_multiplier=-1)
                                nc.tensor.matmul(pot[:], v_sb[:, kc, :],
                                                 e_sb[:], start=(kc == 0),
                                                 stop=(kc == nkc - 1))
                                nc.tensor.matmul(psums[:], ones_col[:],
                                                 e_sb[:], start=(kc == 0),
                                                 stop=(kc == nkc - 1))
                            sums_sb = psm.tile([1, 512], f32, tag="sums",
                                               name="sums_sb")
                            nc.vector.tensor_copy(sums_sb[:], psums[:])
                            recip = psm.tile([1, 512], f32, tag="recip",
                                             name="recip")
                            nc.vector.reciprocal(recip[:], sums_sb[:])
                            recip_r = psm.tile([1, 512], f32r, tag="recipr",
                                               name="recip_r")
                            nc.vector.tensor_copy(recip_r[:], recip[:])
                            pbc = psB.tile([128, 512], f32, tag="pbc",
                                           name="pbc")
                            nc.tensor.matmul(pbc[:], ones_row[:], recip_r[:],
                                             start=True, stop=True)
                            bc_sb = psm.tile([128, 512], f32, tag="bc",
                                             name="bc_sb")
                            nc.vector.tensor_copy(bc_sb[:], pbc[:])
                            ot_sb = pa.tile([128, 512], f32r, tag="ot",
                                            name="ot_sb")
                            nc.vector.tensor_mul(out=ot_sb[:], in0=pot[:],
                                                 in1=bc_sb[:])
                            nc.sync.dma_start(ot_scratch[h, :, ts(qt, 512)],
                                              ot_sb[:])

            # ---- stage 4: out^T = Wo_g^T @ O^T (XT pool closed) ----
            with (
                tc.tile_pool(name="wo", bufs=1) as pwo,
                tc.tile_pool(name="otl", bufs=2) as pot_l,
                tc.tile_pool(name="ost", bufs=2) as post,
            ):
                wo_sb = pwo.tile([128, NHG, D], f32r, name="wo_sb")
                nc.sync.dma_start(
                    wo_sb[:], wo_in.rearrange("(h p) f -> p h f", p=128))
                for st in range(QT_TILES):
                    ot_all = pot_l.tile([128, NHG, 512], f32r, tag="ot_all",
                                        name="ot_all")
                    for h in range(NHG):
                        nc.sync.dma_start(ot_all[:, h, :],
                                          ot_scratch[h, :, ts(st, 512)])
                    for mt in range(D // 128):
                        pp = psA.tile([128, 512], f32, tag="pp", name="pp")
                        for h in range(NHG):
                            nc.tensor.matmul(pp[:], wo_sb[:, h, ts(mt, 128)],
                                             ot_all[:, h, :],
                                             start=(h == 0),
                                             stop=(h == NHG - 1))
                        o_sb = post.tile([128, 512], f32, tag="osb",
                                         name="o_sb")
                        nc.vector.tensor_copy(o_sb[:], pp[:])
                        nc.sync.dma_start(outT[ts(mt, 128), ts(st, 512)],
                                          o_sb[:])

    nc.compile()
    return nc


def _in_maps(x, Wq, Wk, Wv, Wo):
    slopes = np.asarray(_get_slopes(NH), dtype=np.float32)
    pos = np.arange(S, dtype=np.float32)
    dist = np.float32(S - 1) - pos                       # (S,)
    ones_col = np.ones((128, 1), np.float32)
    ones_row = np.ones((1, 128), np.float32)
    ident = np.eye(128, dtype=np.float32)

    in_maps = []
    for b in range(B):
        for g in range(2):
            sl = slopes[g * NHG:(g + 1) * NHG]            # (8,)
            # alibi_k[p, h*DC+kc] = -sl[h] * dist[kc*128+p]
            ak = np.empty((128, NHG * DC), np.float32)
            d2 = dist.reshape(DC, 128)                    # [kc, p]
            for h in range(NHG):
                ak[:, h * DC:(h + 1) * DC] = (-sl[h] * d2).T
            alibi_q = (sl[:, None] * dist[None, :]).astype(np.float32)
            in_maps.append({
                "x": np.ascontiguousarray(x[b]),
                "wq": np.ascontiguousarray(Wq[:, g * NHG * HD:(g + 1) * NHG * HD]),
                "wk": np.ascontiguousarray(Wk[:, g * NHG * HD:(g + 1) * NHG * HD]),
                "wv": np.ascontiguousarray(Wv[:, g * NHG * HD:(g + 1) * NHG * HD]),
                "wo": np.ascontiguousarray(Wo[g * NHG * HD:(g + 1) * NHG * HD, :]),
                "alibi_k": ak,
                "alibi_q": alibi_q,
                "ones_col": ones_col,
                "ones_row": ones_row,
                "ident": ident,
            })
    return in_maps


def kernel(x, Wq, Wk, Wv, Wo, _trace=False):
    from concourse.bass_utils import run_bass_kernel_spmd

    if "nc" not in _cache:
        _cache["nc"] = _build()
    nc = _cache["nc"]

    res = run_bass_kernel_spmd(
        nc, _in_maps(x, Wq, Wk, Wv, Wo), core_ids=list(range(2 * B)),
        trace=_trace)
    _cache["last_exec_time_ns"] = res.exec_time_ns

    out = np.empty((B, S, D), dtype=np.float32)
    for b in range(B):
        out[b] = (res.results[2 * b]["outT"] + res.results[2 * b + 1]["outT"]).T
    return out
